# revision 19
# baseline (speedup 1.0000x reference)
"""Trainium2 Bass kernel for LpAlignEntropyLoss (B=2048, D=128, 2 views).

loss = mean_i ||z0_i - z1_i + eps||  -  0.5 * sum_v mean_i [ logsumexp_{j!=i}(-||zv_i - zv_j + eps||) - log(B-1) ]

Strategy (8 NeuronCores, batch-row sharded, 256 rows/core, symmetric-half):
  dist^2[i,j] = n_i + n_j - 2 * z_i . z_j   (matmul trick, bf16 TensorE)
  The 16x16 grid of 128x128 blocks is covered once using symmetry:
  row-chunk p computes the 9 consecutive column blocks at ring distance
  0..8 (a [128,1152] PSUM slab; the distance-8 block is computed by both
  endpoint chunks, each for its own rows).  exp(-dist) row sums for
  distances 1..7 are also column-summed (stationary-operand matmuls, PE
  cost ~ free size = 1) and shipped to the transposed rows on the host,
  so every row sees all 2047 partners while the ACT engine (the
  bottleneck: 0.833 ns/elem for Sqrt and Exp, dtype-independent) only
  processes 9/16 of the full distance matrix.

  Norm terms are pure matmul tricks against sq = zt*zt (DVE):
   - n_j along the free axis:  lhsT = negh (all -0.5), rhs = sq  ->
     out[m,n] = -0.5 * n_j for every partition m.
   - n_i along partitions: free-size-1 colnorm matmuls -> f32 Sqrt bias.
  Only 5 input DMAs (zt0 in 3 pieces + consts + zt1): HWDGE
  descriptor-gen is ~625 ns and globally serialized, so few DMAs win.

  - diagonal self-masked by accumulating -BIG*I into PSUM (identity matmul)
  - ScalarE pass 1: dist = Sqrt(-2*psum + n_row)  (bias = per-partition
    n_i); the first slab's Sqrt runs in two pieces so ACT starts earlier.
  - ScalarE pass 2: E = Exp(-dist); all Exp passes take their scale from
    an SBUF operand written after every Sqrt output -> the tile scheduler
    cannot interleave Sqrt/Exp, so exactly 2 activation-table loads are
    paid; a dummy early Sqrt pulls the first load into the idle head.
  - row sums via DVE tensor_scalar+accum (2x bf16 mode), column sums via
    stationary matmuls, staging copy on ACT (Copy needs no table load),
    one output DMA.
  - warm-up matmuls into a spare PSUM bank keep PE continuously busy from
    t~1.1us so it reaches the full 2.4 GHz pstate for the real matmuls.
  Host finishes the O(B) tail: assemble rowsums, log, sqrt, means.

eps=1e-8 is below fp32 ulp of every operand magnitude here; dropping it is
exact at fp32 resolution.
"""
import numpy as np
import ml_dtypes
from contextlib import ExitStack

B = 2048
D = 128
N_CORES = 8
R = B // N_CORES          # 256 rows per core
W = 1280                  # local columns held per core (10 chunks)
SLAB = 1152               # slab width (ring distance 0..8)
BIG = float(2 ** 20)
TAU = 1.0
LOG_NM1 = float(np.log(B - 1))
N_WARMUP = 21             # PE pstate warm-up matmuls

_cache: dict = {}


def _build():
    import concourse.tile as tile
    from concourse import bacc, mybir
    import concourse.mybir as mb

    f32 = mybir.dt.float32
    bf16 = mybir.dt.bfloat16
    AF = mybir.ActivationFunctionType

    nc = bacc.Bacc("TRN2", target_bir_lowering=False, debug=False,
                   num_devices=N_CORES)

    zt_d = [nc.dram_tensor(f"zt{v}", [D, W], bf16, kind="ExternalInput").ap()
            for v in (0, 1)]
    consts_d = nc.dram_tensor("consts", [128, 256], bf16,
                              kind="ExternalInput").ap()
    out_d = nc.dram_tensor("out", [128, 34], f32, kind="ExternalOutput").ap()

    with tile.TileContext(nc) as tc, ExitStack() as ctx:
        consts = ctx.enter_context(tc.tile_pool(name="consts", bufs=1))
        ztp = ctx.enter_context(tc.tile_pool(name="ztp", bufs=1))
        psum = ctx.enter_context(tc.tile_pool(name="psum", bufs=1, space="PSUM"))
        distp = ctx.enter_context(tc.tile_pool(name="distp", bufs=1))
        ep = ctx.enter_context(tc.tile_pool(name="ep", bufs=1))
        outp = ctx.enter_context(tc.tile_pool(name="outp", bufs=1))

        # ---- input DMAs on the SP HWDGE queue ----
        sb_zt = []
        for v in (0, 1):
            t_ = ztp.tile([D, W], bf16, tag=f"zt{v}", name=f"sb_zt{v}")
            sb_zt.append(t_)
        sb_c = consts.tile([128, 256], bf16, tag="consts", name="sb_c")
        nc.sync.dma_start(sb_zt[0][:, 0:512], zt_d[0][:, 0:512])
        nc.sync.dma_start(sb_c[:], consts_d)
        nc.sync.dma_start(sb_zt[0][:, 512:1024], zt_d[0][:, 512:1024])
        nc.sync.dma_start(sb_zt[0][:, 1024:1280], zt_d[0][:, 1024:1280])
        nc.sync.dma_start(sb_zt[1][:], zt_d[1])
        ident = sb_c[:, 0:128]
        ibig = sb_c[:, 128:256]

        ones = consts.tile([128, 128], bf16, tag="ones", name="ones")
        nc.vector.memset(ones[:], 1.0)
        negh = consts.tile([128, 128], bf16, tag="negh", name="negh")
        nc.vector.memset(negh[:], -0.5)

        # PSUM: 2 rotating 3-bank slabs + 1-bank outP + 1-bank warm = 8.
        # outP: 0..27 colsums (7 per slab), 28/29 align, 30..33 colnorms.
        outP = psum.tile([128, 34], f32, tag="out", name="outP")
        warm = psum.tile([128, 128], f32, tag="warm", name="warm")

        # PE pstate warm-up
        for _ in range(N_WARMUP):
            nc.tensor.matmul(warm[:], ones[:], ones[:], start=True, stop=True)

        # dummy early Sqrt: the sqrt-table load attaches to its (trivial)
        # waits and runs in the idle head instead of gating the first Sqrt.
        dummy = outp.tile([128, 1], f32, tag="dummy", name="dummy")
        nc.scalar.activation(dummy[:], ones[:, 0:1], AF.Sqrt,
                             bias=0.0, scale=1.0)

        # ---- sq = zt*zt (DVE); n_i colnorms for the f32 Sqrt bias ----
        sq = []
        for v in (0, 1):
            s_ = outp.tile([128, W], bf16, tag=f"sq{v}", name=f"sq{v}")
            sq.append(s_)
        nc.vector.tensor_mul(sq[0][:, 0:512], sb_zt[0][:, 0:512],
                             sb_zt[0][:, 0:512])
        nc.vector.tensor_mul(sq[0][:, 512:1024], sb_zt[0][:, 512:1024],
                             sb_zt[0][:, 512:1024])
        nc.vector.tensor_mul(sq[0][:, 1024:1280], sb_zt[0][:, 1024:1280],
                             sb_zt[0][:, 1024:1280])
        nc.vector.tensor_mul(sq[1][:, 0:1024], sb_zt[1][:, 0:1024],
                             sb_zt[1][:, 0:1024])
        nc.vector.tensor_mul(sq[1][:, 1024:1280], sb_zt[1][:, 1024:1280],
                             sb_zt[1][:, 1024:1280])

        nrow = outp.tile([128, 4], f32, tag="nrow", name="nrow")
        for v in (0, 1):
            for t in range(2):
                nc.tensor.matmul(outP[:, 30 + 2 * v + t:31 + 2 * v + t],
                                 sq[v][:, t * 128:(t + 1) * 128], ones[:, 0:1],
                                 start=True, stop=True)
            nc.vector.tensor_copy(nrow[:, 2 * v:2 * v + 2],
                                  outP[:, 30 + 2 * v:32 + 2 * v])

        # ---- slabs: ring distance 0..8 per (view, chunk) ----
        # PSUM regions: [0:512], [512:1024], [1024:1152]; each gets a z
        # matmul and a -n_j/2 matmul (negh x sq); diag-mask in region 0.
        dists = {}
        for v in (0, 1):
            for t in range(2):
                P = psum.tile([128, SLAB], f32, tag="slab", bufs=2, name="P")
                w0 = t * 128
                lhsT = sb_zt[v][:, t * 128:(t + 1) * 128]
                for a, b in ((0, 512), (512, 1024), (1024, 1152)):
                    wsl = slice(w0 + a, w0 + b)
                    nc.tensor.matmul(P[:, a:b], lhsT, sb_zt[v][:, wsl],
                                     start=True, stop=False)
                    nc.tensor.matmul(P[:, a:b], negh[:], sq[v][:, wsl],
                                     start=False, stop=(a > 0))
                nc.tensor.matmul(P[:, 0:128], ident, ibig,
                                 start=False, stop=True)
                idx = v * 2 + t
                dist = distp.tile([128, SLAB], f32, tag=f"dist{idx}",
                                  name=f"dist{idx}")
                if idx == 0:
                    # split first Sqrt so ACT starts as soon as region 0 is
                    # accumulated
                    nc.scalar.activation(dist[:, 0:512], P[:, 0:512], AF.Sqrt,
                                         bias=nrow[:, 0:1], scale=-2.0)
                    nc.scalar.activation(dist[:, 512:1152], P[:, 512:1152],
                                         AF.Sqrt, bias=nrow[:, 0:1],
                                         scale=-2.0)
                else:
                    nc.scalar.activation(dist[:], P[:], AF.Sqrt,
                                         bias=nrow[:, idx:idx + 1], scale=-2.0)
                dists[idx] = dist

        # ---- align term: ||z0_i - z1_i||^2 for this core's 256 rows ----
        adiff = outp.tile([128, 256], bf16, tag="adiff", name="adiff")
        nc.vector.tensor_sub(adiff[:], sb_zt[0][:, 0:256], sb_zt[1][:, 0:256])
        asq = outp.tile([128, 256], bf16, tag="asq", name="asq")
        nc.vector.tensor_mul(asq[:], adiff[:], adiff[:])
        for h in range(2):
            nc.tensor.matmul(outP[:, 28 + h:29 + h],
                             asq[:, h * 128:(h + 1) * 128], ones[:, 0:1],
                             start=True, stop=True)

        # ---- serialize all Exp after every Sqrt: Exp's scale operand is
        # derived from all Sqrt outputs -> exactly 2 ACT table loads.
        neg = outp.tile([128, 1], f32, tag="neg", name="neg")
        nc.vector.tensor_scalar(neg[:], dists[0][:, 0:1], 0.0, -1.0,
                                mb.AluOpType.mult, mb.AluOpType.add)
        for dep in (dists[0][:, 512:513], dists[1][:, 0:1],
                    dists[2][:, 0:1], dists[3][:, 0:1]):
            nc.vector.tensor_tensor(neg[:], neg[:], dep,
                                    mb.AluOpType.bypass)

        export = outp.tile([128, 34], f32, tag="export", name="export")
        rdump = ep.tile([128, SLAB], bf16, tag="rdump", name="rdump")

        # ---- Exp passes + row sums (DVE accum) + column sums (matmuls) ----
        for idx in range(4):
            E = ep.tile([128, SLAB], bf16, tag=f"e{idx}", name=f"e{idx}")
            nc.scalar.activation(E[:], dists[idx][:], AF.Exp,
                                 scale=neg[:, 0:1])
            nc.vector.tensor_scalar(rdump[:], E[:], 1.0, 0.0,
                                    mb.AluOpType.mult, mb.AluOpType.add,
                                    accum_out=export[:, 30 + idx:31 + idx])
            for b in range(1, 8):
                col = idx * 7 + (b - 1)
                nc.tensor.matmul(outP[:, col:col + 1],
                                 E[:, b * 128:(b + 1) * 128], ones[:, 0:1],
                                 start=True, stop=True)

        # staging copy on ACT (Copy is in every table set: no reload)
        nc.scalar.copy(export[:, 0:30], outP[:, 0:30])
        nc.sync.dma_start(out_d, export[:])

    nc.compile()
    return nc


def _prep_inputs(z0: np.ndarray, z1: np.ndarray):
    """Per-core input maps: rotate columns so core c's rows come first."""
    bf = ml_dtypes.bfloat16
    zs = [np.ascontiguousarray(z0, np.float32), np.ascontiguousarray(z1, np.float32)]
    eye = np.eye(128, dtype=np.float32)
    consts = np.concatenate([eye, -BIG * eye], axis=1).astype(bf)  # [128, 256]
    in_maps = []
    for c in range(N_CORES):
        order = (np.arange(W) + c * R) % B
        m = {"consts": consts}
        for v in (0, 1):
            zr = zs[v][order]                                    # [W, D] rotated
            m[f"zt{v}"] = np.ascontiguousarray(zr.T).astype(bf)  # [D, W]
        in_maps.append(m)
    return in_maps


def kernel(z0: np.ndarray, z1: np.ndarray) -> np.ndarray:
    from concourse.bass_utils import run_bass_kernel_spmd

    if "nc" not in _cache:
        _cache["nc"] = _build()
    nc = _cache["nc"]

    in_maps = _prep_inputs(z0, z1)
    res = run_bass_kernel_spmd(nc, in_maps, core_ids=list(range(N_CORES)))

    rowsums = np.zeros((2, B), np.float64)   # [view, global row]
    alignsq = np.empty((B,), np.float64)
    for c in range(N_CORES):
        out = res.results[c]["out"].astype(np.float64)   # [128, 34]
        for v in (0, 1):
            for t in range(2):
                idx = v * 2 + t
                own = ((2 * c + t) % 16) * 128
                # own row sums cover ring distances 0..8
                rowsums[v, own:own + 128] += out[:, 30 + idx]
                # received column sums (distances 1..7, transposed rows)
                for b in range(1, 8):
                    g = ((2 * c + t + b) % 16) * 128
                    rowsums[v, g:g + 128] += out[:, idx * 7 + (b - 1)]
        alignsq[c * R:c * R + 128] = out[:, 28]
        alignsq[c * R + 128:c * R + 256] = out[:, 29]

    align_loss = np.sqrt(alignsq).mean()
    lme = np.log(rowsums) - LOG_NM1             # [2, B]
    entropy_loss = lme.mean()
    return np.float32(align_loss - entropy_loss)


# revision 23
# speedup vs baseline: 1.0331x; 1.0331x over previous
"""Trainium2 Bass kernel for LpAlignEntropyLoss (B=2048, D=128, 2 views).

loss = mean_i ||z0_i - z1_i + eps||  -  0.5 * sum_v mean_i [ logsumexp_{j!=i}(-||zv_i - zv_j + eps||) - log(B-1) ]

Strategy (8 NeuronCores, batch-row sharded, 256 rows/core, symmetric-half):
  dist^2[i,j] = n_i + n_j - 2 * z_i . z_j   (matmul trick, bf16 TensorE)
  The 16x16 grid of 128x128 blocks is covered once using symmetry:
  row-chunk p computes the 9 consecutive column blocks at ring distance
  0..8 (a [128,1152] PSUM slab; the distance-8 block is computed by both
  endpoint chunks, each for its own rows).  exp(-dist) row sums for
  distances 1..7 are also column-summed (stationary-operand matmuls, PE
  cost ~ free size = 1) and shipped to the transposed rows on the host,
  so every row sees all 2047 partners while the ACT engine (the
  bottleneck: 0.833 ns/elem for Sqrt and Exp, dtype-independent) only
  processes 9/16 of the full distance matrix.

  Norm terms are pure matmul tricks against sq = zt*zt (DVE):
   - n_j along the free axis:  lhsT = negh (all -0.5), rhs = sq  ->
     out[m,n] = -0.5 * n_j for every partition m.
   - n_i along partitions: free-size-1 colnorm matmuls -> f32 Sqrt bias.
  Only 5 input DMAs (zt0 in 3 pieces + consts + zt1): HWDGE
  descriptor-gen is ~625 ns and globally serialized, so few DMAs win.

  - diagonal self-masked by accumulating -BIG*I into PSUM (identity matmul)
  - ScalarE pass 1: dist = Sqrt(-2*psum + n_row)  (bias = per-partition
    n_i); the first slab's Sqrt runs in two pieces so ACT starts earlier.
  - ScalarE pass 2: E = Exp(-dist); all Exp passes take their scale from
    an SBUF operand written after every Sqrt output -> the tile scheduler
    cannot interleave Sqrt/Exp, so exactly 2 activation-table loads are
    paid; a dummy early Sqrt pulls the first load into the idle head.
  - row sums via DVE tensor_scalar+accum (2x bf16 mode), column sums via
    stationary matmuls, staging copy on ACT (Copy needs no table load),
    one output DMA.
  - warm-up matmuls into a spare PSUM bank keep PE continuously busy from
    t~1.1us so it reaches the full 2.4 GHz pstate for the real matmuls.
  Host finishes the O(B) tail: assemble rowsums, log, sqrt, means.

eps=1e-8 is below fp32 ulp of every operand magnitude here; dropping it is
exact at fp32 resolution.
"""
import numpy as np
import ml_dtypes
from contextlib import ExitStack

B = 2048
D = 128
N_CORES = 8
R = B // N_CORES          # 256 rows per core
W = 1280                  # local columns held per core (10 chunks)
SLAB = 1152               # slab width (ring distance 0..8)
BIG = float(2 ** 20)
TAU = 1.0
LOG_NM1 = float(np.log(B - 1))
N_WARMUP = 21             # PE pstate warm-up matmuls

_cache: dict = {}


def _build():
    import concourse.tile as tile
    from concourse import bacc, mybir
    import concourse.mybir as mb

    f32 = mybir.dt.float32
    bf16 = mybir.dt.bfloat16
    AF = mybir.ActivationFunctionType

    nc = bacc.Bacc("TRN2", target_bir_lowering=False, debug=False,
                   num_devices=N_CORES)

    zt_d = [nc.dram_tensor(f"zt{v}", [D, W], bf16, kind="ExternalInput").ap()
            for v in (0, 1)]
    consts_d = nc.dram_tensor("consts", [128, 256], bf16,
                              kind="ExternalInput").ap()
    out_d = nc.dram_tensor("out", [128, 34], f32, kind="ExternalOutput").ap()

    with tile.TileContext(nc) as tc, ExitStack() as ctx:
        consts = ctx.enter_context(tc.tile_pool(name="consts", bufs=1))
        ztp = ctx.enter_context(tc.tile_pool(name="ztp", bufs=1))
        psum = ctx.enter_context(tc.tile_pool(name="psum", bufs=1, space="PSUM"))
        distp = ctx.enter_context(tc.tile_pool(name="distp", bufs=1))
        ep = ctx.enter_context(tc.tile_pool(name="ep", bufs=1))
        outp = ctx.enter_context(tc.tile_pool(name="outp", bufs=1))

        # ---- input DMAs on the SP HWDGE queue ----
        sb_zt = []
        for v in (0, 1):
            t_ = ztp.tile([D, W], bf16, tag=f"zt{v}", name=f"sb_zt{v}")
            sb_zt.append(t_)
        sb_c = consts.tile([128, 256], bf16, tag="consts", name="sb_c")
        nc.sync.dma_start(sb_zt[0][:, 0:1024], zt_d[0][:, 0:1024])
        nc.sync.dma_start(sb_c[:], consts_d)
        nc.sync.dma_start(sb_zt[0][:, 1024:1280], zt_d[0][:, 1024:1280])
        nc.sync.dma_start(sb_zt[1][:], zt_d[1])
        ident = sb_c[:, 0:128]
        ibig = sb_c[:, 128:256]

        ones = consts.tile([128, 128], bf16, tag="ones", name="ones")
        nc.vector.memset(ones[:], 1.0)
        negh = consts.tile([128, 128], bf16, tag="negh", name="negh")
        nc.vector.memset(negh[:], -0.5)

        # PSUM: 2 rotating 3-bank slabs + 1-bank outP + 1-bank warm = 8.
        # outP: 0..27 colsums (7 per slab), 28/29 align, 30..33 colnorms.
        outP = psum.tile([128, 34], f32, tag="out", name="outP")
        warm = psum.tile([128, 128], f32, tag="warm", name="warm")

        # PE pstate warm-up
        for _ in range(N_WARMUP):
            nc.tensor.matmul(warm[:], ones[:], ones[:], start=True, stop=True)

        # dummy early Sqrt: the sqrt-table load attaches to its (trivial)
        # waits and runs in the idle head instead of gating the first Sqrt.
        dummy = outp.tile([128, 1], f32, tag="dummy", name="dummy")
        nc.scalar.activation(dummy[:], ones[:, 0:1], AF.Sqrt,
                             bias=0.0, scale=1.0)

        # ---- sq = zt*zt (DVE); n_i colnorms for the f32 Sqrt bias ----
        sq = []
        for v in (0, 1):
            s_ = outp.tile([128, W], bf16, tag=f"sq{v}", name=f"sq{v}")
            sq.append(s_)
        nc.vector.tensor_mul(sq[0][:, 0:1024], sb_zt[0][:, 0:1024],
                             sb_zt[0][:, 0:1024])
        nc.vector.tensor_mul(sq[0][:, 1024:1280], sb_zt[0][:, 1024:1280],
                             sb_zt[0][:, 1024:1280])
        nc.vector.tensor_mul(sq[1][:, 0:1024], sb_zt[1][:, 0:1024],
                             sb_zt[1][:, 0:1024])
        nc.vector.tensor_mul(sq[1][:, 1024:1280], sb_zt[1][:, 1024:1280],
                             sb_zt[1][:, 1024:1280])

        nrow = outp.tile([128, 4], f32, tag="nrow", name="nrow")
        for v in (0, 1):
            for t in range(2):
                nc.tensor.matmul(outP[:, 30 + 2 * v + t:31 + 2 * v + t],
                                 sq[v][:, t * 128:(t + 1) * 128], ones[:, 0:1],
                                 start=True, stop=True)
            nc.vector.tensor_copy(nrow[:, 2 * v:2 * v + 2],
                                  outP[:, 30 + 2 * v:32 + 2 * v])

        # ---- slabs: ring distance 0..8 per (view, chunk) ----
        # PSUM regions: [0:512], [512:1024], [1024:1152]; each gets a z
        # matmul and a -n_j/2 matmul (negh x sq); diag-mask in region 0.
        dists = {}
        for v in (0, 1):
            for t in range(2):
                P = psum.tile([128, SLAB], f32, tag="slab", bufs=2, name="P")
                w0 = t * 128
                lhsT = sb_zt[v][:, t * 128:(t + 1) * 128]
                for a, b in ((0, 512), (512, 1024), (1024, 1152)):
                    wsl = slice(w0 + a, w0 + b)
                    nc.tensor.matmul(P[:, a:b], lhsT, sb_zt[v][:, wsl],
                                     start=True, stop=False)
                    nc.tensor.matmul(P[:, a:b], negh[:], sq[v][:, wsl],
                                     start=False, stop=(a > 0))
                nc.tensor.matmul(P[:, 0:128], ident, ibig,
                                 start=False, stop=True)
                idx = v * 2 + t
                dist = distp.tile([128, SLAB], f32, tag=f"dist{idx}",
                                  name=f"dist{idx}")
                nc.scalar.activation(dist[:], P[:], AF.Sqrt,
                                     bias=nrow[:, idx:idx + 1], scale=-2.0)
                dists[idx] = dist

        # ---- align term: ||z0_i - z1_i||^2 for this core's 256 rows ----
        adiff = outp.tile([128, 256], bf16, tag="adiff", name="adiff")
        nc.vector.tensor_sub(adiff[:], sb_zt[0][:, 0:256], sb_zt[1][:, 0:256])
        asq = outp.tile([128, 256], bf16, tag="asq", name="asq")
        nc.vector.tensor_mul(asq[:], adiff[:], adiff[:])
        for h in range(2):
            nc.tensor.matmul(outP[:, 28 + h:29 + h],
                             asq[:, h * 128:(h + 1) * 128], ones[:, 0:1],
                             start=True, stop=True)

        # ---- serialize all Exp after every Sqrt: Exp's scale operand (-1)
        # is produced on ACT itself right after the last Sqrt (Copy needs no
        # table load and no cross-engine sem) -> exactly 2 ACT table loads.
        neg = outp.tile([128, 1], f32, tag="neg", name="neg")
        nc.scalar.activation(neg[:], dists[3][:, 0:1], AF.Copy,
                             bias=-1.0, scale=0.0)

        export = outp.tile([128, 34], f32, tag="export", name="export")
        rdump = ep.tile([128, SLAB], bf16, tag="rdump", name="rdump")

        # ---- Exp passes + row sums (DVE accum) + column sums (matmuls) ----
        for idx in range(4):
            E = ep.tile([128, SLAB], bf16, tag=f"e{idx}", name=f"e{idx}")
            nc.scalar.activation(E[:], dists[idx][:], AF.Exp,
                                 scale=neg[:, 0:1])
            nc.vector.tensor_scalar(rdump[:], E[:], 1.0, 0.0,
                                    mb.AluOpType.mult, mb.AluOpType.add,
                                    accum_out=export[:, 30 + idx:31 + idx])
            for b in range(1, 8):
                col = idx * 7 + (b - 1)
                nc.tensor.matmul(outP[:, col:col + 1],
                                 E[:, b * 128:(b + 1) * 128], ones[:, 0:1],
                                 start=True, stop=True)

        # staging copy on ACT (Copy is in every table set: no reload)
        nc.scalar.copy(export[:, 0:30], outP[:, 0:30])
        nc.sync.dma_start(out_d, export[:])

    nc.compile()
    return nc


def _prep_inputs(z0: np.ndarray, z1: np.ndarray):
    """Per-core input maps: rotate columns so core c's rows come first."""
    bf = ml_dtypes.bfloat16
    zs = [np.ascontiguousarray(z0, np.float32), np.ascontiguousarray(z1, np.float32)]
    eye = np.eye(128, dtype=np.float32)
    consts = np.concatenate([eye, -BIG * eye], axis=1).astype(bf)  # [128, 256]
    in_maps = []
    for c in range(N_CORES):
        order = (np.arange(W) + c * R) % B
        m = {"consts": consts}
        for v in (0, 1):
            zr = zs[v][order]                                    # [W, D] rotated
            m[f"zt{v}"] = np.ascontiguousarray(zr.T).astype(bf)  # [D, W]
        in_maps.append(m)
    return in_maps


def kernel(z0: np.ndarray, z1: np.ndarray) -> np.ndarray:
    from concourse.bass_utils import run_bass_kernel_spmd

    if "nc" not in _cache:
        _cache["nc"] = _build()
    nc = _cache["nc"]

    in_maps = _prep_inputs(z0, z1)
    res = run_bass_kernel_spmd(nc, in_maps, core_ids=list(range(N_CORES)))

    rowsums = np.zeros((2, B), np.float64)   # [view, global row]
    alignsq = np.empty((B,), np.float64)
    for c in range(N_CORES):
        out = res.results[c]["out"].astype(np.float64)   # [128, 34]
        for v in (0, 1):
            for t in range(2):
                idx = v * 2 + t
                own = ((2 * c + t) % 16) * 128
                # own row sums cover ring distances 0..8
                rowsums[v, own:own + 128] += out[:, 30 + idx]
                # received column sums (distances 1..7, transposed rows)
                for b in range(1, 8):
                    g = ((2 * c + t + b) % 16) * 128
                    rowsums[v, g:g + 128] += out[:, idx * 7 + (b - 1)]
        alignsq[c * R:c * R + 128] = out[:, 28]
        alignsq[c * R + 128:c * R + 256] = out[:, 29]

    align_loss = np.sqrt(alignsq).mean()
    lme = np.log(rowsums) - LOG_NM1             # [2, B]
    entropy_loss = lme.mean()
    return np.float32(align_loss - entropy_loss)
